# revision 35
# baseline (speedup 1.0000x reference)
"""Trainium2 Bass kernel: batched multi-head cross-attention.

Reference computation (per batch element b):
    q = x @ Wq; k,v = split(context @ Wkv)
    per head: attn = softmax(q k^T / 8); o = attn @ v
    out = concat_heads(o) @ Wo + bo

Sharding: pure data parallel - batch B=8, one batch element per NeuronCore,
no collectives. Transposed dataflow (no on-device transposes):

    QT[d,i]  = Wq^T  @ xT          (lhsT=Wq  natural, rhs=x^T fed from host)
    KT[d,j]  = Wk^T  @ cT
    V[j,d]   = cT^T  @ Wv          (lhsT=cT chunk,    rhs=Wv)
    ST[j,i]  = KT_h^T @ QT_h       (per head, contraction d=64)
    PT       = exp(ST / 8)          (shift-invariant softmax, scores O(6))
    OunT,l   = [V_h | 1]^T @ PT     (ones column gives softmax denom for free)
    OnT      = OunT * (1/l)         (recip straight off the PSUM l-row,
                                     partition-broadcast, one DVE multiply)
    outT     = Wo^T @ OnT + bo

Schedule: a software-pipelined spine. The Scalar engine's EXP stream
(128 x [128,512] tiles, ~72us) and the PE matmul stream (~82us) are the two
near-equal engine floors; the kernel interleaves scores(u) -> EXP(u) ->
PV(u-1) with the remaining projection work (q/k head-pair chunks, V chunks)
as PE filler so both engines stay continuously busy. Warmup matmuls ramp the
PE p-state while the first DMAs land. PSUM: 4 shared [128,512] banks for
scores+projections, 4 x [65,512] banks for PV accumulation; the output
projection reuses freed banks at the tail.
"""

import numpy as np
import ml_dtypes

B, N, M, D = 8, 1024, 1024, 512
H, DH = 8, 64
KC = 4          # 512 contraction -> 4 chunks of 128
IC = 2          # 1024 free dim -> 2 chunks of 512
JC = 8          # 1024 keys -> 8 chunks of 128
N_CORES = 8
N_WARMUP = 20   # PE p-state warmup matmuls issued while input DMAs land

_BF16 = ml_dtypes.bfloat16
_CACHE = {}
LAST_RUN = None  # BassKernelResults of the most recent launch (for test.py)


def _build_nc():
    import concourse.bass as bass
    import concourse.mybir as mybir
    import concourse.tile as tile
    from concourse import bacc

    f32 = mybir.dt.float32
    bf16 = mybir.dt.bfloat16
    Exp = mybir.ActivationFunctionType.Exp

    nc = bacc.Bacc()

    # All inputs host-packed so each tensor loads with ONE wide-row DMA
    # (kc-chunks side by side per partition row: 4-8KB descriptors).
    xt = nc.declare_dram_parameter("xt", [128, KC, N], bf16, isOutput=False)
    ct = nc.declare_dram_parameter("ct", [128, KC, M], bf16, isOutput=False)
    wq = nc.declare_dram_parameter("wq", [128, KC, KC, 128], bf16, isOutput=False)
    wk = nc.declare_dram_parameter("wk", [128, KC, KC, 128], bf16, isOutput=False)
    wv = nc.declare_dram_parameter("wv", [128, KC, D], bf16, isOutput=False)
    wo = nc.declare_dram_parameter("wo", [128, KC, D], bf16, isOutput=False)
    bo = nc.declare_dram_parameter("bo", [D, 1], f32, isOutput=False)
    outT = nc.declare_dram_parameter("outT", [D, N], f32, isOutput=True)

    with tile.TileContext(nc) as tc:
        with (
            tc.tile_pool(name="singles", bufs=1) as singles,
            tc.tile_pool(name="pt", bufs=6) as ptp,
            tc.tile_pool(name="pout", bufs=3) as poutp,
        ):
            def sb(shape, dt, tag):
                return singles.tile(shape, dt, tag=tag, name=tag)

            xt_sb = sb([128, KC, N], bf16, "xt")
            ct_sb = sb([128, KC, M], bf16, "ct")
            wq_sb = sb([128, KC, KC, 128], bf16, "wq")
            wk_sb = sb([128, KC, KC, 128], bf16, "wk")
            wv_sb = sb([128, KC, D], bf16, "wv")
            wo_sb = sb([128, KC, D], bf16, "wo")
            bo_sb = sb([128, KC, 1], f32, "bo")
            qt_sb = [sb([128, N], bf16, f"qt{c}") for c in range(KC)]
            kt_sb = [sb([128, M], bf16, f"kt{c}") for c in range(KC)]
            v_sb = [sb([128, H, DH + 1], bf16, f"v{j}") for j in range(JC)]
            on_sb = [sb([128, N], bf16, f"on{c}") for c in range(KC)]
            lrow = [sb([1, N], f32, f"lrow{h}") for h in range(H)]
            linv = [sb([1, N], f32, f"linv{h}") for h in range(H)]
            r_sb = [sb([128, N], f32, f"r{i}") for i in range(2)]
            jw_a = sb([1, 1], bf16, "jwa")
            jw_b = sb([1, 512], bf16, "jwb")
            jw_exp = sb([1, 512], bf16, "jwe")

            # ---- loads: one wide DMA per tensor, halves striped across
            # the two HWDGE queues; weight tensors for the first head pair
            # gate the spine so they go right after xt/ct halves.
            nc.sync.dma_start(out=xt_sb[:, 0:2, :], in_=xt[:, 0:2, :])
            nc.scalar.dma_start(out=xt_sb[:, 2:4, :], in_=xt[:, 2:4, :])
            nc.sync.dma_start(out=wq_sb, in_=wq[:, :, :, :])
            nc.scalar.dma_start(out=ct_sb[:, 0:2, :], in_=ct[:, 0:2, :])
            nc.sync.dma_start(out=ct_sb[:, 2:4, :], in_=ct[:, 2:4, :])
            nc.scalar.dma_start(out=wk_sb, in_=wk[:, :, :, :])
            nc.sync.dma_start(out=wv_sb[:, 0:2, :], in_=wv[:, 0:2, :])
            nc.scalar.dma_start(out=wv_sb[:, 2:4, :], in_=wv[:, 2:4, :])
            nc.scalar.dma_start(
                out=bo_sb, in_=bo[:, :].rearrange("(c p) o -> p c o", p=128)
            )
            nc.sync.dma_start(out=wo_sb[:, 0:2, :], in_=wo[:, 0:2, :])
            nc.scalar.dma_start(out=wo_sb[:, 2:4, :], in_=wo[:, 2:4, :])
            # junk warmup inputs (no DMA dependency)
            nc.vector.memset(jw_a, 1.0)
            nc.vector.memset(jw_b, 1.0)
            # preload the Exp activation table while DMAs land, so the first
            # real EXP doesn't pay the ~1.5us ACT_TABLE_LOAD mid-spine
            nc.scalar.activation(out=jw_exp, in_=jw_b, func=Exp, scale=0.125)

            with (
                # shared pool: score tiles + projection accumulators
                tc.tile_pool(name="psS", bufs=2, space="PSUM") as psS,
                # PV accumulators [65, 512] (64 head dims + denominator row)
                tc.tile_pool(name="psO", bufs=4, space="PSUM") as psO,
            ):
                def ps_tile():
                    return psS.tile([128, N], f32, tag="psS", name="psS")

                # ---- PE p-state warmup: tiny matmuls with no data deps ----
                wt = ps_tile()
                for _ in range(N_WARMUP):
                    nc.tensor.matmul(
                        wt[0:1, 0:512], lhsT=jw_a, rhs=jw_b,
                        start=True, stop=True, skip_group_check=True,
                    )

                def proj_qk(dst, w_sb, src, dc):
                    """One head-pair [128,1024] q/k projection chunk.
                    kc outer, both ic inner: each stationary block loads once
                    for two 512-col streams (the reload is the expensive
                    part: ~120ns exposed per lhsT switch)."""
                    ps = ps_tile()
                    for kc in range(KC):
                        for ic in range(IC):
                            nc.tensor.matmul(
                                ps[:, ic * 512:(ic + 1) * 512],
                                lhsT=w_sb[:, kc, dc, :],
                                rhs=src[:, kc, ic * 512:(ic + 1) * 512],
                                start=(kc == 0),
                                stop=(kc == KC - 1),
                            )
                    for ic in range(IC):
                        nc.vector.tensor_copy(
                            dst[dc][:, ic * 512:(ic + 1) * 512],
                            ps[:, ic * 512:(ic + 1) * 512],
                        )

                def proj_v(jc):
                    ps = ps_tile()
                    for kc in range(KC):
                        nc.tensor.matmul(
                            ps[:, 0:512],
                            lhsT=ct_sb[:, kc, jc * 128:(jc + 1) * 128],
                            rhs=wv_sb[:, kc, :],
                            start=(kc == 0),
                            stop=(kc == KC - 1),
                        )
                    nc.vector.memset(v_sb[jc][:, :, DH:DH + 1], 1.0)
                    nc.vector.tensor_copy(
                        v_sb[jc][:, :, 0:DH],
                        ps[:, 0:512].rearrange("p (h d) -> p h d", h=H),
                    )

                # ---- prologue: q/k head-pair 0 so the spine can start;
                # casts interleaved by ic so the first scores matmul's two
                # dependencies complete first.
                pro = {}
                for key, w_sb, src_sb in (("q", wq_sb, xt_sb),
                                          ("k", wk_sb, ct_sb)):
                    ps = ps_tile()
                    pro[key] = ps
                    for kc in range(KC):
                        for ic in range(IC):
                            nc.tensor.matmul(
                                ps[:, ic * 512:(ic + 1) * 512],
                                lhsT=w_sb[:, kc, 0, :],
                                rhs=src_sb[:, kc, ic * 512:(ic + 1) * 512],
                                start=(kc == 0),
                                stop=(kc == KC - 1),
                            )
                for ic in range(IC):
                    for key, dst in (("q", qt_sb), ("k", kt_sb)):
                        nc.vector.tensor_copy(
                            dst[0][:, ic * 512:(ic + 1) * 512],
                            pro[key][:, ic * 512:(ic + 1) * 512],
                        )

                # filler schedule: unit index -> list of closures.  V chunk jc
                # is consumed by the (lag-2) PV at unit jc+2; q/k pair dc is
                # consumed by the scores at unit 16*dc.
                fillers = {}
                for jc in range(JC):
                    fillers.setdefault(jc, []).append(
                        lambda jc=jc: proj_v(jc))
                for dc in range(1, KC):
                    # 2 groups per dc ahead of the 16*dc due date; both land
                    # BEFORE the norm chain that enters the DVE queue at unit
                    # 16*dc - 7 (else the filler casts queue behind the
                    # ~4.6us norm chain and stall the next scores matmul).
                    slots = [16 * dc - 10, 16 * dc - 8]
                    for sl, (w_sb, src, dst) in zip(slots, (
                        (wq_sb, xt_sb, qt_sb), (wk_sb, ct_sb, kt_sb)
                    )):
                        fillers.setdefault(sl, []).append(
                            lambda w=w_sb, s=src, d=dst, dc=dc:
                                proj_qk(d, w, s, dc))

                Copy = mybir.ActivationFunctionType.Copy

                def norm(h):
                    """1/l from the PSUM denominator row, broadcast, scale.
                    For the last pair the l-row extraction goes on the (by
                    then idle) Scalar engine to shorten the tail chain."""
                    dc, pb = h // 2, (h % 2) * 64
                    for ic in range(IC):
                        if h >= 7:
                            nc.scalar.activation(
                                out=lrow[h][:, ic * 512:(ic + 1) * 512],
                                in_=pso_tiles[(h, ic)][DH:DH + 1, :],
                                func=Copy,
                            )
                        else:
                            nc.vector.tensor_copy(
                                lrow[h][:, ic * 512:(ic + 1) * 512],
                                pso_tiles[(h, ic)][DH:DH + 1, :],
                            )
                    nc.vector.reciprocal_approx_fast(out=linv[h], in_=lrow[h])
                    nc.gpsimd.partition_broadcast(r_sb[h % 2], linv[h][0:1, :])
                    for ic in range(IC):
                        nc.vector.tensor_mul(
                            on_sb[dc][pb:pb + 64, ic * 512:(ic + 1) * 512],
                            pso_tiles[(h, ic)][0:DH, :],
                            r_sb[h % 2][0:DH, ic * 512:(ic + 1) * 512],
                        )

                # ---- the spine: 64 units of (scores pair -> EXP, PV lagged
                # two units behind, filler).  Lag 2 keeps every PE wait
                # pre-satisfied as long as the Scalar engine holds its pace.
                pso_tiles = {}
                pending = []
                for u in range(64):
                    h, jc = u // 8, u % 8
                    dc, pb = h // 2, (h % 2) * 64
                    pss = ps_tile()
                    for ic in range(IC):
                        nc.tensor.matmul(
                            pss[:, ic * 512:(ic + 1) * 512],
                            lhsT=kt_sb[dc][pb:pb + 64, jc * 128:(jc + 1) * 128],
                            rhs=qt_sb[dc][pb:pb + 64, ic * 512:(ic + 1) * 512],
                            start=True,
                            stop=True,
                        )
                    ptile = ptp.tile([128, N], bf16, tag="pt", name="pt")
                    nc.scalar.activation(out=ptile, in_=pss, func=Exp, scale=0.125)

                    def pv(h=h, jc=jc, ptile=ptile):
                        if jc == 0:
                            for ic in range(IC):
                                pso_tiles[(h, ic)] = psO.tile(
                                    [DH + 1, 512], f32, tag="psO", name="psO"
                                )
                        for ic in range(IC):
                            nc.tensor.matmul(
                                pso_tiles[(h, ic)],
                                lhsT=v_sb[jc][:, h, :],
                                rhs=ptile[:, ic * 512:(ic + 1) * 512],
                                start=(jc == 0),
                                stop=(jc == JC - 1),
                            )
                        if jc == JC - 1:
                            norm(h)
                    pending.append(pv)
                    if len(pending) > 2:
                        pending.pop(0)()
                    for f in fillers.pop(u, ()):
                        f()
                for f in pending:
                    f()

                # ---- output projection + bias (Scalar), stores striped.
                # First two row-blocks accumulate head pairs 0-2 while the
                # last pair's normalization chain drains, then finish with
                # hc=3; the PE never has to sit idle waiting for on_sb[3].
                Identity = mybir.ActivationFunctionType.Identity

                def e_finish(ec, ps):
                    for ic in range(IC):
                        nc.tensor.matmul(
                            ps[:, ic * 512:(ic + 1) * 512],
                            lhsT=wo_sb[:, KC - 1, ec * 128:(ec + 1) * 128],
                            rhs=on_sb[KC - 1][:, ic * 512:(ic + 1) * 512],
                            start=False,
                            stop=True,
                        )
                        ot = poutp.tile([128, 512], f32, tag="pout", name="pout")
                        if ec < 2:   # drain biases on both engines in parallel
                            nc.scalar.activation(
                                out=ot, in_=ps[:, ic * 512:(ic + 1) * 512],
                                func=Identity, bias=bo_sb[:, ec, :], scale=1.0,
                            )
                        else:
                            nc.vector.tensor_scalar_add(
                                ot, ps[:, ic * 512:(ic + 1) * 512],
                                bo_sb[:, ec, :],
                            )
                        q = nc.sync if (ec + ic) % 2 == 0 else nc.scalar
                        q.dma_start(
                            out=outT[ec * 128:(ec + 1) * 128,
                                     ic * 512:(ic + 1) * 512],
                            in_=ot,
                        )

                def e_partial(ec, ps, hc_end):
                    for hc in range(hc_end):
                        for ic in range(IC):
                            nc.tensor.matmul(
                                ps[:, ic * 512:(ic + 1) * 512],
                                lhsT=wo_sb[:, hc, ec * 128:(ec + 1) * 128],
                                rhs=on_sb[hc][:, ic * 512:(ic + 1) * 512],
                                start=(hc == 0),
                                stop=False,
                            )

                e_tiles = {}
                for ec in range(2):
                    e_tiles[ec] = ps_tile()
                    e_partial(ec, e_tiles[ec], KC - 1)
                for ec in range(2):
                    e_finish(ec, e_tiles[ec])
                for ec in range(2, KC):
                    ps = ps_tile()
                    e_partial(ec, ps, KC - 1)
                    e_finish(ec, ps)
    nc.finalize()
    return nc


def _ensure_ntff_hook():
    """Install antenv.axon_hooks if the image lacks it, registering the
    ctypes NTFF-profile hook against libaxon_pjrt.so. Without this,
    run_bass_kernel_spmd(trace=True)/BASS_TRACE=1 crashes on import."""
    import contextlib
    import ctypes
    import os
    import sys
    import types

    try:
        import antenv.axon_hooks  # noqa: F401
        return
    except ImportError:
        pass
    try:
        import antenv
    except ImportError:
        return

    state = {"hook": None}
    mod = types.ModuleType("antenv.axon_hooks")
    mod.set_axon_ntff_profile_hook = lambda h: state.__setitem__("hook", h)
    mod.get_axon_ntff_profile_hook = lambda: state["hook"]
    sys.modules["antenv.axon_hooks"] = mod
    antenv.axon_hooks = mod

    so_path = "/opt/axon/libaxon_pjrt.so"
    if not os.path.exists(so_path):
        return
    try:
        lib = ctypes.CDLL(so_path)
    except OSError:
        return
    if not hasattr(lib, "axon_start_nrt_profile"):
        return
    lib.axon_start_nrt_profile.argtypes = [
        ctypes.POINTER(ctypes.c_int64), ctypes.c_size_t,
    ]
    lib.axon_start_nrt_profile.restype = ctypes.c_int64
    lib.axon_stop_nrt_profile.argtypes = [ctypes.c_char_p]
    lib.axon_stop_nrt_profile.restype = ctypes.c_int64

    @contextlib.contextmanager
    def _hook(output_dir, device_ids):
        import jax
        jax.devices()  # force PJRT init so the .so's client exists
        if device_ids:
            ids = (ctypes.c_int64 * len(device_ids))(*device_ids)
            rc = lib.axon_start_nrt_profile(ids, len(device_ids))
        else:
            rc = lib.axon_start_nrt_profile(None, 0)
        if rc != 0:
            raise RuntimeError(f"axon_start_nrt_profile rc={rc}")
        try:
            yield
        finally:
            n = lib.axon_stop_nrt_profile(str(output_dir).encode())
            if n <= 0:
                print(f"ntff profile: rc={n} (no profile output)")

    state["hook"] = _hook


def kernel(x, context, Wq, Wkv, Wo, bo):
    global LAST_RUN
    _ensure_ntff_hook()
    from concourse import bass_utils

    if "nc" not in _CACHE:
        _CACHE["nc"] = _build_nc()
    nc = _CACHE["nc"]

    def pack_kc(a):  # [KC*128, F] -> [128, KC, F] (kc chunks side by side)
        return np.ascontiguousarray(
            a.reshape(KC, 128, -1).transpose(1, 0, 2))

    wq4 = np.ascontiguousarray(Wq, dtype=np.float32).astype(_BF16)
    wk4 = np.ascontiguousarray(Wkv[:, :D], dtype=np.float32).astype(_BF16)
    # [p, kc, dc, j] so the whole tensor loads with one 4KB-row DMA
    wq_ = np.ascontiguousarray(wq4.reshape(KC, 128, KC, 128).transpose(1, 0, 2, 3))
    wk_ = np.ascontiguousarray(wk4.reshape(KC, 128, KC, 128).transpose(1, 0, 2, 3))
    wv_ = pack_kc(np.ascontiguousarray(Wkv[:, D:], np.float32).astype(_BF16))
    wo_ = pack_kc(np.ascontiguousarray(Wo, dtype=np.float32).astype(_BF16))
    bo_ = np.ascontiguousarray(np.asarray(bo, dtype=np.float32).reshape(D, 1))

    in_maps = []
    for b in range(B):
        m = {
            "xt": pack_kc(np.asarray(x[b], np.float32).T.astype(_BF16)),
            "ct": pack_kc(np.asarray(context[b], np.float32).T.astype(_BF16)),
            "wq": wq_, "wk": wk_, "wv": wv_, "wo": wo_, "bo": bo_,
        }
        in_maps.append(m)

    LAST_RUN = bass_utils.run_bass_kernel_spmd(nc, in_maps, list(range(N_CORES)))
    out = np.empty((B, N, D), dtype=np.float32)
    for b in range(B):
        out[b] = LAST_RUN.results[b]["outT"].T
    return out


# revision 36
# speedup vs baseline: 1.2035x; 1.2035x over previous
"""Trainium2 Bass kernel: batched multi-head cross-attention.

Reference computation (per batch element b):
    q = x @ Wq; k,v = split(context @ Wkv)
    per head: attn = softmax(q k^T / 8); o = attn @ v
    out = concat_heads(o) @ Wo + bo

Sharding: pure data parallel - batch B=8, one batch element per NeuronCore,
no collectives. Transposed dataflow (no on-device transposes):

    QT[d,i]  = Wq^T  @ xT          (lhsT=Wq  natural, rhs=x^T fed from host)
    KT[d,j]  = Wk^T  @ cT
    V[j,d]   = cT^T  @ Wv          (lhsT=cT chunk,    rhs=Wv)
    ST[j,i]  = KT_h^T @ QT_h       (per head, contraction d=64)
    PT       = exp(ST / 8)          (shift-invariant softmax, scores O(6))
    OunT,l   = [V_h | 1]^T @ PT     (ones column gives softmax denom for free)
    OnT      = OunT * (1/l)         (recip straight off the PSUM l-row,
                                     partition-broadcast, one DVE multiply)
    outT     = Wo^T @ OnT + bo

Schedule: a software-pipelined spine. The Scalar engine's EXP stream
(128 x [128,512] tiles, ~72us) and the PE matmul stream (~82us) are the two
near-equal engine floors; the kernel interleaves scores(u) -> EXP(u) ->
PV(u-1) with the remaining projection work (q/k head-pair chunks, V chunks)
as PE filler so both engines stay continuously busy. Warmup matmuls ramp the
PE p-state while the first DMAs land. PSUM: 4 shared [128,512] banks for
scores+projections, 4 x [65,512] banks for PV accumulation; the output
projection reuses freed banks at the tail.
"""

import numpy as np
import ml_dtypes

B, N, M, D = 8, 1024, 1024, 512
H, DH = 8, 64
KC = 4          # 512 contraction -> 4 chunks of 128
IC = 2          # 1024 free dim -> 2 chunks of 512
JC = 8          # 1024 keys -> 8 chunks of 128
N_CORES = 8
N_WARMUP = 20   # PE p-state warmup matmuls issued while input DMAs land

_BF16 = ml_dtypes.bfloat16
_CACHE = {}
LAST_RUN = None  # BassKernelResults of the most recent launch (for test.py)


def _build_nc():
    import concourse.bass as bass
    import concourse.mybir as mybir
    import concourse.tile as tile
    from concourse import bacc

    f32 = mybir.dt.float32
    bf16 = mybir.dt.bfloat16
    Exp = mybir.ActivationFunctionType.Exp

    nc = bacc.Bacc()

    # All inputs host-packed so each tensor loads with ONE wide-row DMA
    # (kc-chunks side by side per partition row: 4-8KB descriptors).
    xt = nc.declare_dram_parameter("xt", [128, KC, N], bf16, isOutput=False)
    ct = nc.declare_dram_parameter("ct", [128, KC, M], bf16, isOutput=False)
    wq = nc.declare_dram_parameter("wq", [128, KC, KC, 128], bf16, isOutput=False)
    wk = nc.declare_dram_parameter("wk", [128, KC, KC, 128], bf16, isOutput=False)
    wv = nc.declare_dram_parameter("wv", [128, KC, D], bf16, isOutput=False)
    wo = nc.declare_dram_parameter("wo", [128, KC, D], bf16, isOutput=False)
    bo = nc.declare_dram_parameter("bo", [D, 1], f32, isOutput=False)
    outT = nc.declare_dram_parameter("outT", [D, N], f32, isOutput=True)

    with tile.TileContext(nc) as tc:
        with (
            tc.tile_pool(name="singles", bufs=1) as singles,
            tc.tile_pool(name="pt", bufs=6) as ptp,
            tc.tile_pool(name="pout", bufs=3) as poutp,
        ):
            def sb(shape, dt, tag):
                return singles.tile(shape, dt, tag=tag, name=tag)

            xt_sb = sb([128, KC, N], bf16, "xt")
            ct_sb = sb([128, KC, M], bf16, "ct")
            wq_sb = sb([128, KC, KC, 128], bf16, "wq")
            wk_sb = sb([128, KC, KC, 128], bf16, "wk")
            wv_sb = sb([128, KC, D], bf16, "wv")
            wo_sb = sb([128, KC, D], bf16, "wo")
            bo_sb = sb([128, KC, 1], f32, "bo")
            qt_sb = [sb([128, N], bf16, f"qt{c}") for c in range(KC)]
            kt_sb = [sb([128, M], bf16, f"kt{c}") for c in range(KC)]
            v_sb = [sb([128, H, DH + 1], bf16, f"v{j}") for j in range(JC)]
            on_sb = [sb([128, N], bf16, f"on{c}") for c in range(KC)]
            lrow = [sb([1, N], f32, f"lrow{h}") for h in range(H)]
            linv = [sb([1, N], f32, f"linv{h}") for h in range(H)]
            r_sb = [sb([128, N], f32, f"r{i}") for i in range(2)]
            jw_a = sb([1, 1], bf16, "jwa")
            jw_b = sb([1, 512], bf16, "jwb")
            jw_exp = sb([1, 512], bf16, "jwe")

            # ---- loads: one wide DMA per tensor, halves striped across
            # the two HWDGE queues; weight tensors for the first head pair
            # gate the spine so they go right after xt/ct halves.
            nc.sync.dma_start(out=xt_sb[:, 0:2, :], in_=xt[:, 0:2, :])
            nc.scalar.dma_start(out=xt_sb[:, 2:4, :], in_=xt[:, 2:4, :])
            nc.sync.dma_start(out=wq_sb, in_=wq[:, :, :, :])
            nc.scalar.dma_start(out=ct_sb[:, 0:2, :], in_=ct[:, 0:2, :])
            nc.sync.dma_start(out=ct_sb[:, 2:4, :], in_=ct[:, 2:4, :])
            nc.scalar.dma_start(out=wk_sb, in_=wk[:, :, :, :])
            nc.sync.dma_start(out=wv_sb[:, 0:2, :], in_=wv[:, 0:2, :])
            nc.scalar.dma_start(out=wv_sb[:, 2:4, :], in_=wv[:, 2:4, :])
            nc.scalar.dma_start(
                out=bo_sb, in_=bo[:, :].rearrange("(c p) o -> p c o", p=128)
            )
            nc.sync.dma_start(out=wo_sb[:, 0:2, :], in_=wo[:, 0:2, :])
            nc.scalar.dma_start(out=wo_sb[:, 2:4, :], in_=wo[:, 2:4, :])
            # junk warmup inputs (no DMA dependency)
            nc.vector.memset(jw_a, 1.0)
            nc.vector.memset(jw_b, 1.0)
            # preload the Exp activation table while DMAs land, so the first
            # real EXP doesn't pay the ~1.5us ACT_TABLE_LOAD mid-spine
            nc.scalar.activation(out=jw_exp, in_=jw_b, func=Exp, scale=0.125)

            with (
                # shared pool: score tiles + projection accumulators
                tc.tile_pool(name="psS", bufs=2, space="PSUM") as psS,
                # PV accumulators [65, 512] (64 head dims + denominator row)
                tc.tile_pool(name="psO", bufs=4, space="PSUM") as psO,
            ):
                def ps_tile():
                    return psS.tile([128, N], f32, tag="psS", name="psS")

                # ---- PE p-state warmup: tiny matmuls with no data deps ----
                wt = ps_tile()
                for _ in range(N_WARMUP):
                    nc.tensor.matmul(
                        wt[0:1, 0:512], lhsT=jw_a, rhs=jw_b,
                        start=True, stop=True, skip_group_check=True,
                    )

                def proj_qk(dst, w_sb, src, dc):
                    """One head-pair [128,1024] q/k projection chunk.
                    kc outer, both ic inner: each stationary block loads once
                    for two 512-col streams (the reload is the expensive
                    part: ~120ns exposed per lhsT switch)."""
                    ps = ps_tile()
                    for kc in range(KC):
                        for ic in range(IC):
                            nc.tensor.matmul(
                                ps[:, ic * 512:(ic + 1) * 512],
                                lhsT=w_sb[:, kc, dc, :],
                                rhs=src[:, kc, ic * 512:(ic + 1) * 512],
                                start=(kc == 0),
                                stop=(kc == KC - 1),
                            )
                    for ic in range(IC):
                        nc.vector.tensor_copy(
                            dst[dc][:, ic * 512:(ic + 1) * 512],
                            ps[:, ic * 512:(ic + 1) * 512],
                        )

                def proj_v(jc):
                    ps = ps_tile()
                    for kc in range(KC):
                        nc.tensor.matmul(
                            ps[:, 0:512],
                            lhsT=ct_sb[:, kc, jc * 128:(jc + 1) * 128],
                            rhs=wv_sb[:, kc, :],
                            start=(kc == 0),
                            stop=(kc == KC - 1),
                        )
                    nc.vector.memset(v_sb[jc][:, :, DH:DH + 1], 1.0)
                    nc.vector.tensor_copy(
                        v_sb[jc][:, :, 0:DH],
                        ps[:, 0:512].rearrange("p (h d) -> p h d", h=H),
                    )

                # ---- prologue: q/k head-pair 0 so the spine can start;
                # casts interleaved by ic so the first scores matmul's two
                # dependencies complete first.
                pro = {}
                for key, w_sb, src_sb in (("q", wq_sb, xt_sb),
                                          ("k", wk_sb, ct_sb)):
                    ps = ps_tile()
                    pro[key] = ps
                    for kc in range(KC):
                        for ic in range(IC):
                            nc.tensor.matmul(
                                ps[:, ic * 512:(ic + 1) * 512],
                                lhsT=w_sb[:, kc, 0, :],
                                rhs=src_sb[:, kc, ic * 512:(ic + 1) * 512],
                                start=(kc == 0),
                                stop=(kc == KC - 1),
                            )
                for ic in range(IC):
                    for key, dst in (("q", qt_sb), ("k", kt_sb)):
                        nc.vector.tensor_copy(
                            dst[0][:, ic * 512:(ic + 1) * 512],
                            pro[key][:, ic * 512:(ic + 1) * 512],
                        )

                # filler schedule: unit index -> list of closures.  V chunk jc
                # is consumed by the (lag-2) PV at unit jc+2; q/k pair dc is
                # consumed by the scores at unit 16*dc.
                fillers = {}
                for jc in range(JC):
                    fillers.setdefault(jc, []).append(
                        lambda jc=jc: proj_v(jc))
                for dc in range(1, KC):
                    # 2 groups per dc ahead of the 16*dc due date; both land
                    # BEFORE the norm chain that enters the DVE queue at unit
                    # 16*dc - 7 (else the filler casts queue behind the
                    # ~4.6us norm chain and stall the next scores matmul).
                    slots = [16 * dc - 10, 16 * dc - 8]
                    for sl, (w_sb, src, dst) in zip(slots, (
                        (wq_sb, xt_sb, qt_sb), (wk_sb, ct_sb, kt_sb)
                    )):
                        fillers.setdefault(sl, []).append(
                            lambda w=w_sb, s=src, d=dst, dc=dc:
                                proj_qk(d, w, s, dc))

                Copy = mybir.ActivationFunctionType.Copy

                def norm(h):
                    """1/l from the PSUM denominator row, broadcast, scale.
                    The last head's chain gates the output projection, so it
                    runs split by 512-col halves pipelined across Scalar
                    (l-row ic0), DVE (l-row ic1, recips, mults) and GpSimd
                    (broadcasts): ~4.2us instead of ~5.9us serial."""
                    dc, pb = h // 2, (h % 2) * 64
                    if h == 7:
                        nc.scalar.activation(
                            out=lrow[h][:, 0:512],
                            in_=pso_tiles[(h, 0)][DH:DH + 1, :], func=Copy,
                        )
                        nc.vector.tensor_copy(
                            lrow[h][:, 512:1024],
                            pso_tiles[(h, 1)][DH:DH + 1, :],
                        )
                        for ic in range(IC):
                            nc.vector.reciprocal_approx_fast(
                                out=linv[h][:, ic * 512:(ic + 1) * 512],
                                in_=lrow[h][:, ic * 512:(ic + 1) * 512],
                            )
                        for ic in range(IC):
                            nc.gpsimd.partition_broadcast(
                                r_sb[h % 2][:, ic * 512:(ic + 1) * 512],
                                linv[h][0:1, ic * 512:(ic + 1) * 512],
                            )
                        for ic in range(IC):
                            nc.vector.tensor_mul(
                                on_sb[dc][pb:pb + 64, ic * 512:(ic + 1) * 512],
                                pso_tiles[(h, ic)][0:DH, :],
                                r_sb[h % 2][0:DH, ic * 512:(ic + 1) * 512],
                            )
                        return
                    for ic in range(IC):
                        nc.vector.tensor_copy(
                            lrow[h][:, ic * 512:(ic + 1) * 512],
                            pso_tiles[(h, ic)][DH:DH + 1, :],
                        )
                    nc.vector.reciprocal_approx_fast(out=linv[h], in_=lrow[h])
                    nc.gpsimd.partition_broadcast(r_sb[h % 2], linv[h][0:1, :])
                    for ic in range(IC):
                        nc.vector.tensor_mul(
                            on_sb[dc][pb:pb + 64, ic * 512:(ic + 1) * 512],
                            pso_tiles[(h, ic)][0:DH, :],
                            r_sb[h % 2][0:DH, ic * 512:(ic + 1) * 512],
                        )

                # ---- the spine: 64 units of (scores pair -> EXP, PV lagged
                # two units behind, filler).  Lag 2 keeps every PE wait
                # pre-satisfied as long as the Scalar engine holds its pace.
                pso_tiles = {}
                pending = []
                for u in range(64):
                    h, jc = u // 8, u % 8
                    dc, pb = h // 2, (h % 2) * 64
                    pss = ps_tile()
                    for ic in range(IC):
                        nc.tensor.matmul(
                            pss[:, ic * 512:(ic + 1) * 512],
                            lhsT=kt_sb[dc][pb:pb + 64, jc * 128:(jc + 1) * 128],
                            rhs=qt_sb[dc][pb:pb + 64, ic * 512:(ic + 1) * 512],
                            start=True,
                            stop=True,
                        )
                    ptile = ptp.tile([128, N], bf16, tag="pt", name="pt")
                    nc.scalar.activation(out=ptile, in_=pss, func=Exp, scale=0.125)

                    def pv(h=h, jc=jc, ptile=ptile):
                        if jc == 0:
                            for ic in range(IC):
                                pso_tiles[(h, ic)] = psO.tile(
                                    [DH + 1, 512], f32, tag="psO", name="psO"
                                )
                        for ic in range(IC):
                            nc.tensor.matmul(
                                pso_tiles[(h, ic)],
                                lhsT=v_sb[jc][:, h, :],
                                rhs=ptile[:, ic * 512:(ic + 1) * 512],
                                start=(jc == 0),
                                stop=(jc == JC - 1),
                            )
                        if jc == JC - 1:
                            norm(h)
                    pending.append(pv)
                    if len(pending) > 2:
                        pending.pop(0)()
                    for f in fillers.pop(u, ()):
                        f()
                for f in pending:
                    f()

                # ---- output projection + bias (Scalar), stores striped.
                # First two row-blocks accumulate head pairs 0-2 while the
                # last pair's normalization chain drains, then finish with
                # hc=3; the PE never has to sit idle waiting for on_sb[3].
                Identity = mybir.ActivationFunctionType.Identity

                def e_finish(ec, ps):
                    for ic in range(IC):
                        nc.tensor.matmul(
                            ps[:, ic * 512:(ic + 1) * 512],
                            lhsT=wo_sb[:, KC - 1, ec * 128:(ec + 1) * 128],
                            rhs=on_sb[KC - 1][:, ic * 512:(ic + 1) * 512],
                            start=False,
                            stop=True,
                        )
                        ot = poutp.tile([128, 512], f32, tag="pout", name="pout")
                        if ec < 2:   # drain biases on both engines in parallel
                            nc.scalar.activation(
                                out=ot, in_=ps[:, ic * 512:(ic + 1) * 512],
                                func=Identity, bias=bo_sb[:, ec, :], scale=1.0,
                            )
                        else:
                            nc.vector.tensor_scalar_add(
                                ot, ps[:, ic * 512:(ic + 1) * 512],
                                bo_sb[:, ec, :],
                            )
                        q = nc.sync if (ec + ic) % 2 == 0 else nc.scalar
                        q.dma_start(
                            out=outT[ec * 128:(ec + 1) * 128,
                                     ic * 512:(ic + 1) * 512],
                            in_=ot,
                        )

                def e_partial(ec, ps, hc_end):
                    for hc in range(hc_end):
                        for ic in range(IC):
                            nc.tensor.matmul(
                                ps[:, ic * 512:(ic + 1) * 512],
                                lhsT=wo_sb[:, hc, ec * 128:(ec + 1) * 128],
                                rhs=on_sb[hc][:, ic * 512:(ic + 1) * 512],
                                start=(hc == 0),
                                stop=False,
                            )

                e_tiles = {}
                for ec in range(2):
                    e_tiles[ec] = ps_tile()
                    e_partial(ec, e_tiles[ec], KC - 1)
                for ec in range(2):
                    e_finish(ec, e_tiles[ec])
                for ec in range(2, KC):
                    ps = ps_tile()
                    e_partial(ec, ps, KC - 1)
                    e_finish(ec, ps)
    nc.finalize()
    return nc


def _ensure_ntff_hook():
    """Install antenv.axon_hooks if the image lacks it, registering the
    ctypes NTFF-profile hook against libaxon_pjrt.so. Without this,
    run_bass_kernel_spmd(trace=True)/BASS_TRACE=1 crashes on import."""
    import contextlib
    import ctypes
    import os
    import sys
    import types

    try:
        import antenv.axon_hooks  # noqa: F401
        return
    except ImportError:
        pass
    try:
        import antenv
    except ImportError:
        return

    state = {"hook": None}
    mod = types.ModuleType("antenv.axon_hooks")
    mod.set_axon_ntff_profile_hook = lambda h: state.__setitem__("hook", h)
    mod.get_axon_ntff_profile_hook = lambda: state["hook"]
    sys.modules["antenv.axon_hooks"] = mod
    antenv.axon_hooks = mod

    so_path = "/opt/axon/libaxon_pjrt.so"
    if not os.path.exists(so_path):
        return
    try:
        lib = ctypes.CDLL(so_path)
    except OSError:
        return
    if not hasattr(lib, "axon_start_nrt_profile"):
        return
    lib.axon_start_nrt_profile.argtypes = [
        ctypes.POINTER(ctypes.c_int64), ctypes.c_size_t,
    ]
    lib.axon_start_nrt_profile.restype = ctypes.c_int64
    lib.axon_stop_nrt_profile.argtypes = [ctypes.c_char_p]
    lib.axon_stop_nrt_profile.restype = ctypes.c_int64

    @contextlib.contextmanager
    def _hook(output_dir, device_ids):
        import jax
        jax.devices()  # force PJRT init so the .so's client exists
        if device_ids:
            ids = (ctypes.c_int64 * len(device_ids))(*device_ids)
            rc = lib.axon_start_nrt_profile(ids, len(device_ids))
        else:
            rc = lib.axon_start_nrt_profile(None, 0)
        if rc != 0:
            raise RuntimeError(f"axon_start_nrt_profile rc={rc}")
        try:
            yield
        finally:
            n = lib.axon_stop_nrt_profile(str(output_dir).encode())
            if n <= 0:
                print(f"ntff profile: rc={n} (no profile output)")

    state["hook"] = _hook


def kernel(x, context, Wq, Wkv, Wo, bo):
    global LAST_RUN
    _ensure_ntff_hook()
    from concourse import bass_utils

    if "nc" not in _CACHE:
        _CACHE["nc"] = _build_nc()
    nc = _CACHE["nc"]

    def pack_kc(a):  # [KC*128, F] -> [128, KC, F] (kc chunks side by side)
        return np.ascontiguousarray(
            a.reshape(KC, 128, -1).transpose(1, 0, 2))

    wq4 = np.ascontiguousarray(Wq, dtype=np.float32).astype(_BF16)
    wk4 = np.ascontiguousarray(Wkv[:, :D], dtype=np.float32).astype(_BF16)
    # [p, kc, dc, j] so the whole tensor loads with one 4KB-row DMA
    wq_ = np.ascontiguousarray(wq4.reshape(KC, 128, KC, 128).transpose(1, 0, 2, 3))
    wk_ = np.ascontiguousarray(wk4.reshape(KC, 128, KC, 128).transpose(1, 0, 2, 3))
    wv_ = pack_kc(np.ascontiguousarray(Wkv[:, D:], np.float32).astype(_BF16))
    wo_ = pack_kc(np.ascontiguousarray(Wo, dtype=np.float32).astype(_BF16))
    bo_ = np.ascontiguousarray(np.asarray(bo, dtype=np.float32).reshape(D, 1))

    in_maps = []
    for b in range(B):
        m = {
            "xt": pack_kc(np.asarray(x[b], np.float32).T.astype(_BF16)),
            "ct": pack_kc(np.asarray(context[b], np.float32).T.astype(_BF16)),
            "wq": wq_, "wk": wk_, "wv": wv_, "wo": wo_, "bo": bo_,
        }
        in_maps.append(m)

    LAST_RUN = bass_utils.run_bass_kernel_spmd(nc, in_maps, list(range(N_CORES)))
    out = np.empty((B, N, D), dtype=np.float32)
    for b in range(B):
        out[b] = LAST_RUN.results[b]["outT"].T
    return out
